# revision 45
# baseline (speedup 1.0000x reference)
"""Trainium2 Bass kernel: 9-pattern masked depthwise 3x3 conv, 2 branches.

Full problem: xh, xl [4, 16, 512, 512] fp32; wh, wl, mh, ml [9, 16, 3, 3].
out = stack([conv9(xh, wh*mh), conv9(xl, wl*ml)])  -> [2, 9, 4, 16, 510, 510]
with clamp(-128, 127) and round-half-even applied elementwise.

Sharding: pure data parallel over (branch, batch) = 8 independent slices,
one per NeuronCore. No cross-core communication.

Per-core kernel strategy (row-blocked matmul + vector-engine assist,
3 matmuls/row):
  - Output rows are processed in 85 blocks of 6. Per block one DMA loads the
    8 input rows it needs as x8[(a,c), w] (partition = a*16+c, a = row
    offset 0..7) -- each input row lands in SBUF once (1.33x total reads).
  - 6 accumulating matmul chains per block (M=128, K=128, N=510) produce
    output row r0+m for patterns 0..7 (M = k*16+c); each chain is 3 fp32r
    matmuls (dj = 0,1,2) whose rhs is the same x8 tile shifted by dj in
    the free dim; di lives in the block-diagonal lhsT (partition a =
    r_off + di). Pattern 8 never touches the PE: its 9 taps run as
    per-partition-scalar FMAs (scalar_tensor_tensor) on the otherwise-
    idle DVE and GPSIMD engines -- in the (c,r) output layout the
    operand partition map is affine (out q=c*8+r reads x8 partition
    q+di), so acc = x8[q+di, j+dj]*w8[c,di,dj] + acc chains directly.
    18 matmuls * 510 cycles per block; DVE/GPSIMD split the width so
    every engine finishes inside the PE's block time.
  - Post-processing is ONE instruction per chain: the hardware's fp32 ->
    int8 dtype conversion rounds half-even AND saturates to [-128, 127],
    so a plain Copy activation (ACT) from PSUM to an int8 SBUF tile
    implements round+clamp+convert exactly (the last pattern-8 FMA
    writes int8 directly the same way).
  - int8 results DMA to HBM in (k c) row-major order, 6 contiguous rows
    per partition (3060B descriptors = full DMA bandwidth); the host
    up-converts to fp32 losslessly.
  - fp32r sacrifices ~11 mantissa bits in the matmul operands, flipping
    ~0.4% of outputs by +-1 at round boundaries (rel l2 err ~1.5e-3);
    use_f32r=False gives exact-fp32 at ~4x the time.
"""

import numpy as np

import concourse.bacc as bacc
import concourse.mybir as mybir
from concourse.tile import TileContext
from concourse.bass_utils import run_bass_kernel_spmd

B, C, H, W = 4, 16, 512, 512
HO, WO = H - 2, W - 2
R = 6            # output rows per block
A = R + 2        # input rows per block
NBLK = HO // R   # 85

MAGIC = 12582912.0  # 1.5 * 2**23 = 192 * 2**16: fp32 RNE round magic
F32 = mybir.dt.float32
F32R = mybir.dt.float32r
I8 = mybir.dt.int8
ADD = mybir.AluOpType.add
COPY = mybir.ActivationFunctionType.Copy

MULT = mybir.AluOpType.mult
CW = 254   # pattern-8 split: DVE FMA cols [0, CW), PE matmul cols [CW, WO)
PW = WO - CW  # 256: PE's pattern-8 width (>= 256 keeps fp32r at 1 cy/row)
# di-major so the three aligned di=0 taps run while the shifted views land
TAPS = [(di, dj) for di in (0, 1, 2) for dj in (0, 1, 2)]


def _shift_mask(di):
    """stream_shuffle mask: within each 32-partition quadrant (= 4 channels
    x 8 row-slots of the x8 tile), shift the row-slot index down by di so
    out[c*8+r] = x8[c*8+r+di] (clamped; slots r>5 are unused garbage)."""
    return [(i // 8) * 8 + min(i % 8 + di, 7) for i in range(32)]

_CACHE = {}


def _build_nc(use_f32r=True, reps=1):
    nc = bacc.Bacc()
    mmdt = F32R if use_f32r else F32

    x = nc.declare_dram_parameter("x", [C, H, W], F32, isOutput=False)
    lw = nc.declare_dram_parameter("lw", [3, 128, 7 * 128], F32, isOutput=False)
    w8 = nc.declare_dram_parameter("w8", [128, len(TAPS)], F32, isOutput=False)
    y = nc.declare_dram_parameter("y", [9, C, HO, WO], I8, isOutput=True)

    with TileContext(nc) as tc:
        with (
            tc.tile_pool(name="lwp", bufs=1) as lwp,
            tc.tile_pool(name="xp", bufs=4) as xp,
            tc.tile_pool(name="outp", bufs=2) as outp,
            tc.tile_pool(name="psm", bufs=1, space="PSUM") as psp,
        ):
            lwt = lwp.tile([128, 3, 7 * 128], mmdt)

            # PE p-state warmup: the Tensor engine ramps to full clock only
            # after ~3us of continuous execution; these dependency-free dummy
            # matmuls run while the first DMAs are in flight so every real
            # matmul is issued against a fully-ramped engine.
            warm = lwp.tile([128, 64], mmdt, tag="warm")
            nc.vector.memset(warm[:].bitcast(F32), 0)
            psw = psp.tile([64, 64], F32, tag="warm")
            for _i in range(32):
                nc.tensor.matmul(
                    psw[:, 0:64], lhsT=warm[:, 0:64], rhs=warm[:, 0:64],
                    start=True, stop=True,
                )

            nblk = NBLK * reps
            x8s = {}
            xs2s = {}

            def load_x8(b):
                blk = b % NBLK
                t = xp.tile([128, W], mmdt, tag="x8", name=f"x8_{b}")
                nc.sync.dma_start(
                    out=t[:],
                    in_=x[:, R * blk : R * blk + A, :].bitcast(mmdt),
                )
                x8s[b] = t

            def shift_x2(b, bootstrap=False):
                # di=2 row-shifted view built by a partition-remapping
                # SBUF->SBUF DMA: xs2[c*8+r] = x8[c*8+r+2] for r < 6. The
                # gap partitions (r = 6,7) keep their bootstrap memset.
                t = xp.tile([128, CW + 2], F32, tag="xs2", name=f"xs2_{b}")
                if bootstrap:
                    nc.vector.memset(t[:], 0)
                nc.sync.dma_start(
                    out=t[:].rearrange("(c a) w -> c a w", c=C)[:, 0:R, :],
                    in_=x8s[b][:, 0 : CW + 2]
                    .bitcast(F32)
                    .rearrange("(c a) w -> c a w", c=C)[:, 2:A, :],
                )
                xs2s[b] = t

            # first input block, then per-dj weight slices, then more input
            # prefetch -- ordered so the first matmul chain's operands arrive
            # as early as possible while the PE warmup is still running.
            load_x8(0)
            s6 = lwp.tile([128, len(TAPS)], F32, tag="s6")
            nc.sync.dma_start(out=s6[:], in_=w8[:])
            for _dj in range(3):
                nc.sync.dma_start(
                    out=lwt[:, _dj, :], in_=lw[_dj].bitcast(mmdt)
                )
            for _pb in range(1, min(3, nblk)):
                load_x8(_pb)
            for _pb in range(min(2, nblk)):
                shift_x2(_pb, bootstrap=True)

            NP8 = C * 8 - 2  # 126: partitions q = c*8 + r (r<6 used)
            for b in range(nblk):
                blk = b % NBLK
                r0 = R * blk
                x8 = x8s.pop(b)
                om = outp.tile([128, R, WO], I8, tag="om", name=f"om_{b}")
                om8 = outp.tile([128, WO], I8, tag="om8", name=f"om8_{b}")

                # Pattern-8 cols [0, CW) run on DVE as 9 FMA taps (the affine
                # (c,r) layout: out q=c*8+r reads row q+di). Engine APs must
                # be partition-aligned, so the row-shifted views are
                # materialized: di=1 with stream_shuffle (in-quadrant
                # permute on DVE), di=2 with a SBUF->SBUF DMA. Cols
                # [CW, WO) stay on the PE (3 matmuls, N=256) with the int8
                # post on the otherwise-idle GPSIMD.
                xs1 = outp.tile([128, CW + 2], F32, tag="xs1", name=f"xs_{b}_1")
                nc.vector.stream_shuffle(
                    xs1[:], x8[:, 0 : CW + 2].bitcast(F32), _shift_mask(1)
                )
                xsh = [x8, xs1, xs2s.pop(b)]
                acc_prev = None
                for t, (di, dj) in enumerate(TAPS):
                    last = t == len(TAPS) - 1
                    if last:
                        dst = om8[0:NP8, 0:CW]
                    else:
                        acc = outp.tile(
                            [128, CW], F32, tag=f"accd{t % 2}",
                            name=f"acc_{b}_{t}",
                        )
                        dst = acc[0:NP8, :]
                    src = xsh[di]
                    in0 = src[0:NP8, dj : CW + dj]
                    if di == 0:
                        in0 = in0.bitcast(F32)
                    sc = s6[0:NP8, t : t + 1]
                    if t == 0:
                        nc.vector.tensor_scalar(dst, in0, sc, None, MULT)
                    else:
                        nc.vector.scalar_tensor_tensor(
                            dst, in0=in0, scalar=sc, in1=acc_prev[0:NP8, :],
                            op0=MULT, op1=ADD,
                        )
                    if not last:
                        acc_prev = acc

                pm6 = psp.tile([128, 256], F32, tag="ps6", name=f"pm6_{b}")
                for dj in range(3):
                    nc.tensor.matmul(
                        pm6[:, 0:PW],
                        lhsT=lwt[:, dj, 768:896],
                        rhs=x8[:, CW + dj : CW + dj + PW],
                        start=(dj == 0),
                        stop=(dj == 2),
                    )
                nc.gpsimd.tensor_scalar(
                    om8[0:NP8, CW:WO], pm6[0:NP8, 0:PW], 0.0, None, ADD
                )

                for m in range(R):
                    pm = psp.tile([128, 512], F32, tag=f"ps{m}", name=f"pm_{b}_{m}")
                    for dj in range(3):
                        nc.tensor.matmul(
                            pm[:, 0:WO],
                            lhsT=lwt[:, dj, 128 * m : 128 * (m + 1)],
                            rhs=x8[:, dj : dj + WO],
                            start=(dj == 0),
                            stop=(dj == 2),
                        )
                    nc.scalar.activation(
                        om[:, m, :], pm[:, 0:WO], COPY, bias=0.0, scale=1.0
                    )
                if b + 3 < nblk:
                    load_x8(b + 3)
                if b + 2 < nblk:
                    shift_x2(b + 2)
                nc.sync.dma_start(
                    out=y[0:8, :, r0 : r0 + R, :],
                    in_=om[:],
                )
                nc.sync.dma_start(
                    out=y[8, :, r0 : r0 + R, :],
                    in_=om8[:].rearrange("(c a) w -> c a w", c=C)[:, 0:R, :],
                )
    return nc


def _host_lw(wm):
    """wm = (w*m) [9, 16, 3, 3] fp32 -> lhsT blocks [3, 128, 896].

    Partition row = c*8 + a (a = input-row offset in the 8-row block).
    Chain m in 0..5: col 128m + k*16 + c = wm[k, c, a-m, dj] (patterns 0..7
    of output row r0+m). Chain 6: col 768 + c*8 + r = wm[8, c, a-r, dj]
    (pattern 8, PE columns [CW, WO) only; DVE covers the rest, _host_w8)."""
    lw = np.zeros((3, 128, 7 * 128), np.float32)
    ks = np.arange(8)
    for dj in range(3):
        for di in range(3):
            for c in range(C):
                for m in range(R):
                    lw[dj, c * 8 + m + di, 128 * m + ks * 16 + c] = wm[:8, c, di, dj]
                for r in range(R):
                    lw[dj, c * 8 + r + di, 768 + c * 8 + r] = wm[8, c, di, dj]
    return lw


def _host_w8(wm):
    """Per-partition FMA scalars for the engine-side pattern-8 taps:
    w8[c*8 + r, t] = wm[8, c, di_t, dj_t] (same weight for every row r)."""
    w8 = np.zeros((128, len(TAPS)), np.float32)
    for t, (di, dj) in enumerate(TAPS):
        for c in range(C):
            w8[c * 8 : c * 8 + R, t] = wm[8, c, di, dj]
    return w8


def _get_nc(use_f32r=True, reps=1):
    key = ("nc", use_f32r, reps)
    if key not in _CACHE:
        nc_new = _build_nc(use_f32r, reps)
        nc_new.finalize()
        _CACHE[key] = nc_new
    return _CACHE[key]


def _in_maps(xh, xl, wh, wl, mh, ml):
    xh = np.ascontiguousarray(np.asarray(xh, dtype=np.float32))
    xl = np.ascontiguousarray(np.asarray(xl, dtype=np.float32))
    wmh = (np.asarray(wh, np.float32) * np.asarray(mh, np.float32)).astype(np.float32)
    wml = (np.asarray(wl, np.float32) * np.asarray(ml, np.float32)).astype(np.float32)
    maps = []
    for x_all, wm_b in [(xh, wmh), (xl, wml)]:
        lw_b = _host_lw(wm_b)
        w8_b = _host_w8(wm_b)
        for b in range(B):
            maps.append({"x": np.ascontiguousarray(x_all[b]), "lw": lw_b, "w8": w8_b})
    return maps


def _finish(y_i8):
    return y_i8.astype(np.float32)


def kernel(xh, xl, wh, wl, mh, ml, h=0, use_f32r=True):
    nc = _get_nc(use_f32r)
    in_maps = _in_maps(xh, xl, wh, wl, mh, ml)
    res = run_bass_kernel_spmd(nc, in_maps, list(range(8)))

    out = np.empty((2, 9, B, C, HO, WO), dtype=np.float32)
    for core, rmap in enumerate(res.results):
        br, b = divmod(core, B)
        out[br, :, b] = _finish(rmap["y"])
    return out


def timed_run(xh, xl, wh, wl, mh, ml, h=0, use_f32r=True, iters=5):
    """Returns (out, best_exec_ns): times the sharded PJRT execution with
    device-resident inputs (transfers excluded via pre-device_put)."""
    import jax, time
    from jax.sharding import Mesh, PartitionSpec, NamedSharding
    from concourse import bass2jax, mybir as _mb

    nc = _get_nc(use_f32r)
    in_maps = _in_maps(xh, xl, wh, wl, mh, ml)
    n_cores = 8
    bass2jax.install_neuronx_cc_hook()
    if nc.dbg_addr is not None and not nc.dbg_callbacks:
        in_maps = [
            {**m, nc.dbg_addr.name: np.zeros((1, 2), np.uint32)} for m in in_maps
        ]
    partition_name = nc.partition_id_tensor.name if nc.partition_id_tensor else None
    in_names, out_names, out_avals, zero_outs = [], [], [], []
    for alloc in nc.m.functions[0].allocations:
        if not isinstance(alloc, _mb.MemoryLocationSet):
            continue
        name = alloc.memorylocations[0].name
        if alloc.kind == "ExternalInput":
            if name != partition_name:
                in_names.append(name)
        elif alloc.kind == "ExternalOutput":
            shape = tuple(alloc.tensor_shape)
            dtype = _mb.dt.np(alloc.dtype)
            out_names.append(name)
            out_avals.append(jax.core.ShapedArray(shape, dtype))
            zero_outs.append(np.zeros(shape, dtype))
    n_params = len(in_names)
    n_outs = len(out_avals)
    in_names_all = in_names + out_names
    if partition_name is not None:
        in_names_all.append(partition_name)
    donate = tuple(range(n_params, n_params + n_outs))

    def _body(*args):
        operands = list(args)
        if partition_name is not None:
            operands.append(bass2jax.partition_id_tensor())
        return tuple(
            bass2jax._bass_exec_p.bind(
                *operands,
                out_avals=tuple(out_avals),
                in_names=tuple(in_names_all),
                out_names=tuple(out_names),
                lowering_input_output_aliases=(),
                sim_require_finite=True,
                sim_require_nnan=True,
                nc=nc,
            )
        )

    devices = jax.devices()[:n_cores]
    mesh = Mesh(np.asarray(devices), ("core",))
    from jax.experimental.shard_map import shard_map
    in_specs = (PartitionSpec("core"),) * (n_params + n_outs)
    out_specs = (PartitionSpec("core"),) * n_outs
    sharded = jax.jit(
        shard_map(_body, mesh=mesh, in_specs=in_specs, out_specs=out_specs,
                  check_rep=False),
        donate_argnums=donate, keep_unused=True,
    )
    sh = NamedSharding(mesh, PartitionSpec("core"))
    concat_in = [
        jax.device_put(
            np.concatenate([np.asarray(in_maps[c][nm]) for c in range(n_cores)], axis=0),
            sh,
        )
        for nm in in_names
    ]
    best = None
    out_arrs = None
    for _ in range(max(1, iters)):
        concat_zeros = [
            jax.device_put(np.zeros((n_cores * z.shape[0], *z.shape[1:]), z.dtype), sh)
            for z in zero_outs
        ]
        jax.block_until_ready(concat_zeros)
        t0 = time.perf_counter_ns()
        out_arrs = sharded(*concat_in, *concat_zeros)
        jax.block_until_ready(out_arrs)
        t1 = time.perf_counter_ns()
        if best is None or t1 - t0 < best:
            best = t1 - t0
    out = np.empty((2, 9, B, C, HO, WO), dtype=np.float32)
    arr = np.asarray(out_arrs[0]).reshape(n_cores, 9, C, HO, WO)
    for core in range(n_cores):
        br, b = divmod(core, B)
        out[br, :, b] = _finish(arr[core])
    return out, best


if __name__ == "__main__":
    rng = np.random.RandomState(0)
    ins = {
        "xh": rng.randn(B, C, H, W).astype(np.float32) * 20,
        "xl": rng.randn(B, C, H, W).astype(np.float32) * 20,
        "wh": rng.randn(9, C, 3, 3).astype(np.float32),
        "wl": rng.randn(9, C, 3, 3).astype(np.float32),
        "mh": np.round(rng.rand(9, C, 3, 3)).astype(np.float32),
        "ml": np.round(rng.rand(9, C, 3, 3)).astype(np.float32),
        "h": 0,
    }
    out = kernel(**ins)
    print("kernel out:", out.shape, out.dtype, out.min(), out.max())
